# revision 2
# baseline (speedup 1.0000x reference)
"""Trainium2 Bass kernel for nn_Blur3d (4x4 separable blur, pad=(2,1)).

Math: reference does a true 2D convolution of each (h, w) = (128, 128) slice
with the normalized 4x4 blur kernel K2 (an outer product of a 1D 4-tap
kernel), zero-padded by (2, 1) on each spatial dim.  For output index i:
    out[i, j] = sum_{bh, bw} K2[bh, bw] * x[i + 1 - bh, j + 1 - bw]
With K2 = outer(kc, kr) this is z = Wc^T @ x @ Wr where Wc/Wr are 128x128
banded matrices: W[j, i] = k[i + 1 - j] (0 <= i + 1 - j < 4).

On-device (per image, x as [h=128 partitions, w=128 free]):
    mm1: u = matmul(lhsT=x,  rhs=Wc)  ->  u = x^T Wc   [w part, ho free]
    mm2: z = matmul(lhsT=u,  rhs=Wr)  ->  z = Wc^T x Wr [ho part, wo free]
The lhsT.T semantics of the tensor engine absorb the inter-pass transpose,
so the output lands in the correct orientation with zero explicit
transposes.  The banded matrices encode the zero padding exactly.

Wire format (default "bf16"): x is cast to bf16 on host and shipped in a
pre-transposed [h, n, w] layout so every DMA is 128 partitions x 8 KiB
contiguous; the band-matrix taps are exactly representable in bf16; both
matmuls run at full bf16 PE rate with fp32 PSUM accumulation; the inter-pass
u tile and the output are written as bf16 (copy-with-cast on DVE / Scalar).
The host upcasts the bf16 result to fp32.  End-to-end max rel err vs the
fp32 reference is ~6e-3 (quantization of x, u, z at 8 mantissa bits each),
while HBM traffic and PE time are both halved+ vs the fp32 path.

Sharding: 4096 independent images, 512 per core, pure data parallel on the
collapsed (batch, c, t) dim across 8 NeuronCores.
"""

import numpy as np

_P = 128          # image height/width and partition count
_NCORES = 8
_CHUNK = 32       # images per DMA chunk
_GROUP = 4        # images per PSUM bank (4 * 128 fp32 = one 2 KiB bank)

# Default build configuration (can be overridden for experiments).
_CFG = dict(wire="bf16", z_copy="scalar", u_copy="vector", chunk=_CHUNK)

_PROGRAM_CACHE = {}
LAST_RESULTS = None  # BassKernelResults of the most recent run (for profiling)


def _taps_from_kernel2d(k2d):
    """Rank-1 (separable) decomposition of the 4x4 blur kernel."""
    k2d = np.asarray(k2d, dtype=np.float64)
    U, S, Vt = np.linalg.svd(k2d)
    kc = U[:, 0] * np.sqrt(S[0])
    kr = Vt[0] * np.sqrt(S[0])
    if kc.sum() < 0.0:
        kc, kr = -kc, -kr
    resid = np.abs(np.outer(kc, kr) - k2d).max()
    if resid > 1e-9 * max(1.0, np.abs(k2d).max()):
        raise ValueError(f"blur kernel is not separable (rank-1 resid {resid})")
    return kc, kr


def _band(taps, n=_P):
    """W[j, i] = taps[i + 1 - j]; encodes conv taps + zero padding."""
    taps = np.asarray(taps, dtype=np.float64)
    kh = taps.shape[0]
    W = np.zeros((n, n), dtype=np.float32)
    for b in range(kh):
        off = 1 - b  # input row j = i + 1 - b
        d = np.float32(taps[b])
        idx_i = np.arange(n)
        idx_j = idx_i + off
        m = (idx_j >= 0) & (idx_j < n)
        W[idx_j[m], idx_i[m]] = d
    return W


def _rep(it, repeats):
    for _ in range(repeats):
        yield from it


def _build_program(n_imgs, chunk=_CHUNK, group=_GROUP, repeats=1, wire="bf16",
                   z_copy="scalar", u_copy="vector"):
    """Build the per-core Bass program.

    wire: "bf16" (x/u/z all bf16, fp32 PSUM accumulate; x arrives host-
          transposed as [h, n, w] bf16, out leaves as [ho, n, wo] bf16)
          or "fp32" (everything fp32; x as [n, h, w], device-side rearrange).
    z_copy/u_copy: engine for the PSUM->SBUF copies ("vector" or "scalar").
    """
    from contextlib import ExitStack

    import concourse.tile as tile
    from concourse import bacc, mybir

    FP = mybir.dt.float32
    BF = mybir.dt.bfloat16
    DT = BF if wire == "bf16" else FP
    nc = bacc.Bacc("TRN2", target_bir_lowering=False, debug=False)

    if wire == "bf16":
        # host-pretransposed: x[h, n, w], out[ho, n, wo] -> contiguous DMA
        x = nc.declare_dram_parameter("x", [_P, n_imgs, _P], BF, isOutput=False)
        out = nc.declare_dram_parameter("out", [_P, n_imgs, _P], BF, isOutput=True)
        wcr = nc.declare_dram_parameter("wcr", [_P, 2, _P], BF, isOutput=False)
    else:
        x = nc.declare_dram_parameter("x", [n_imgs, _P, _P], FP, isOutput=False)
        out = nc.declare_dram_parameter("out", [n_imgs, _P, _P], FP, isOutput=True)
        wcr = nc.declare_dram_parameter("wcr", [_P, 2, _P], FP, isOutput=False)

    assert n_imgs % chunk == 0 and chunk % group == 0

    with tile.TileContext(nc) as tc, ExitStack() as ctx:
        wp = ctx.enter_context(tc.tile_pool(name="w", bufs=1))
        xp = ctx.enter_context(tc.tile_pool(name="x", bufs=3))
        up = ctx.enter_context(tc.tile_pool(name="u", bufs=4))
        op = ctx.enter_context(tc.tile_pool(name="o", bufs=3))
        pu = ctx.enter_context(tc.tile_pool(name="pu", bufs=3, space="PSUM"))
        pz = ctx.enter_context(tc.tile_pool(name="pz", bufs=3, space="PSUM"))
        psc = ctx.enter_context(tc.tile_pool(name="psc", bufs=1, space="PSUM"))

        wt = wp.tile([_P, 2, _P], DT)
        nc.sync.dma_start(wt[:], wcr[:])
        wct = wt[:, 0, :]
        wrt = wt[:, 1, :]

        # dummy matmul absorbs the weight-DMA wait on PE
        scr = psc.tile([_P, 1], FP)
        nc.tensor.matmul(scr[:], lhsT=wct, rhs=wt[:, 0, 0:1], start=True, stop=True)

        engines = {"vector": nc.vector.tensor_copy, "scalar": nc.scalar.copy}
        z_eng = engines[z_copy]
        u_eng = engines[u_copy]

        for c in _rep(range(n_imgs // chunk), repeats):
            xt = xp.tile([_P, chunk, _P], DT)
            if wire == "bf16":
                nc.sync.dma_start(xt[:], x[:, c * chunk : (c + 1) * chunk, :])
            else:
                nc.sync.dma_start(
                    xt[:], x[c * chunk : (c + 1) * chunk].rearrange("i h w -> h i w")
                )
            ot = op.tile([_P, chunk, _P], DT)
            # 1-element DVE touch: absorbs the out-DMA slot-recycle wait so
            # the first z-copy of the chunk carries only its PE wait.
            nc.vector.memset(ot[:, 0, 0:1], 0.0)
            for g in range(chunk // group):
                put = pu.tile([_P, group, _P], FP)
                for j in range(group):
                    i = g * group + j
                    nc.tensor.matmul(put[:, j, :], lhsT=xt[:, i, :],
                                     rhs=wct, start=True, stop=True)
                ust = up.tile([_P, group, _P], DT)
                u_eng(ust[:], put[:])
                pzt = pz.tile([_P, group, _P], FP)
                for j in range(group):
                    nc.tensor.matmul(
                        pzt[:, j, :],
                        lhsT=ust[:, j, :],
                        rhs=wrt,
                        start=True,
                        stop=True,
                    )
                z_eng(ot[:, g * group : (g + 1) * group, :], pzt[:])
            if wire == "bf16":
                nc.sync.dma_start(out[:, c * chunk : (c + 1) * chunk, :], ot[:])
            else:
                nc.sync.dma_start(
                    out[c * chunk : (c + 1) * chunk].rearrange("i h w -> h i w"),
                    ot[:],
                )

    nc.compile()
    return nc


def _get_program(n_imgs, cfg):
    key = (n_imgs, cfg["chunk"], _GROUP, cfg["wire"], cfg["z_copy"], cfg["u_copy"])
    if key not in _PROGRAM_CACHE:
        _PROGRAM_CACHE[key] = _build_program(
            n_imgs, chunk=cfg["chunk"], wire=cfg["wire"], z_copy=cfg["z_copy"],
            u_copy=cfg["u_copy"],
        )
    return _PROGRAM_CACHE[key]


def _prepare_in_maps(input, kernel, cfg):
    """Shard + pack FULL inputs into per-core in_maps for the Bass program."""
    import ml_dtypes

    x = np.asarray(input, dtype=np.float32)
    imgs = x.reshape(-1, _P, _P)
    n = imgs.shape[0]
    assert n % _NCORES == 0
    per = n // _NCORES

    kc, kr = _taps_from_kernel2d(kernel)
    Wc = _band(kc)
    Wr = _band(kr)

    if cfg["wire"] == "bf16":
        bf = ml_dtypes.bfloat16
        wcr_f = np.stack([Wc, Wr], axis=1)  # [j, 2, i]
        if not np.all(wcr_f.astype(bf).astype(np.float32) == wcr_f):
            raise ValueError("blur taps not exact in bf16")
        wcr = np.ascontiguousarray(wcr_f.astype(bf))
        xs = imgs.astype(bf)
        in_maps = [
            {
                "x": np.ascontiguousarray(
                    xs[i * per : (i + 1) * per].transpose(1, 0, 2)
                ),
                "wcr": wcr,
            }
            for i in range(_NCORES)
        ]
    else:
        wcr = np.ascontiguousarray(np.stack([Wc, Wr], axis=1))
        in_maps = [
            {"x": np.ascontiguousarray(imgs[i * per : (i + 1) * per]), "wcr": wcr}
            for i in range(_NCORES)
        ]
    return in_maps, per


def _assemble_output(outs, orig_shape, cfg):
    """Per-core device outputs -> FULL fp32 output of the original shape."""
    if cfg["wire"] == "bf16":
        # each out is [ho, per, wo] bf16
        full = np.concatenate(
            [np.asarray(o).transpose(1, 0, 2) for o in outs], axis=0
        ).astype(np.float32)
    else:
        full = np.concatenate([np.asarray(o) for o in outs], axis=0)
    return np.ascontiguousarray(full).reshape(orig_shape)


def kernel(input, kernel, _trace=False, _cfg=None):
    global LAST_RESULTS
    from concourse.bass_utils import run_bass_kernel_spmd

    cfg = dict(_CFG)
    if _cfg:
        cfg.update(_cfg)

    orig_shape = np.asarray(input).shape
    in_maps, per = _prepare_in_maps(input, kernel, cfg)
    nc = _get_program(per, cfg)
    res = run_bass_kernel_spmd(
        nc, in_maps, core_ids=list(range(_NCORES)), trace=_trace
    )
    LAST_RESULTS = res
    outs = [res.results[i]["out"] for i in range(_NCORES)]
    return _assemble_output(outs, orig_shape, cfg)


# revision 14
# speedup vs baseline: 1.1252x; 1.1252x over previous
"""Trainium2 Bass kernel for nn_Blur3d (4x4 separable blur, pad=(2,1)).

Math: reference does a true 2D convolution of each (h, w) = (128, 128) slice
with the normalized 4x4 blur kernel K2 (an outer product of a 1D 4-tap
kernel), zero-padded by (2, 1) on each spatial dim.  For output index i:
    out[i, j] = sum_{bh, bw} K2[bh, bw] * x[i + 1 - bh, j + 1 - bw]
With K2 = outer(kc, kr) this is z = Wc^T @ x @ Wr where Wc/Wr are 128x128
banded matrices: W[j, i] = k[i + 1 - j] (0 <= i + 1 - j < 4).

On-device (per image, x as [h=128 partitions, w=128 free]):
    mm1: u = matmul(lhsT=x,  rhs=Wc)  ->  u = x^T Wc   [w part, ho free]
    mm2: z = matmul(lhsT=u,  rhs=Wr)  ->  z = Wc^T x Wr [ho part, wo free]
The lhsT.T semantics of the tensor engine absorb the inter-pass transpose,
so the output lands in the correct orientation with zero explicit
transposes.  The banded matrices encode the zero padding exactly.

Wire format (default "bf16"): x is cast to bf16 on host and shipped in a
pre-transposed [h, n, w] layout so every DMA is 128 partitions x 8 KiB
contiguous; the band-matrix taps are exactly representable in bf16; both
matmuls run at full bf16 PE rate with fp32 PSUM accumulation; the inter-pass
u tile and the output are written as bf16 (copy-with-cast on DVE / Scalar).
The host upcasts the bf16 result to fp32.  End-to-end max rel err vs the
fp32 reference is ~6e-3 (quantization of x, u, z at 8 mantissa bits each),
while HBM traffic and PE time are both halved+ vs the fp32 path.

Sharding: 4096 independent images, 512 per core, pure data parallel on the
collapsed (batch, c, t) dim across 8 NeuronCores.
"""

import numpy as np

_P = 128          # image height/width and partition count
_NCORES = 8
_CHUNK = 32       # images per DMA chunk
_GROUP = 4        # images per PSUM bank (4 * 128 fp32 = one 2 KiB bank)

# Default build configuration (can be overridden for experiments).
_CFG = dict(wire="bf16", z_copy="scalar+vector", u_copy="vector+scalar",
            chunk=64, dma_in="sync", dma_out="sync", xbufs=3, obufs=3,
            algo="phased", pubufs=2, cgroup=8)

_PROGRAM_CACHE = {}
LAST_RESULTS = None  # BassKernelResults of the most recent run (for profiling)


def _taps_from_kernel2d(k2d):
    """Rank-1 (separable) decomposition of the 4x4 blur kernel."""
    k2d = np.asarray(k2d, dtype=np.float64)
    U, S, Vt = np.linalg.svd(k2d)
    kc = U[:, 0] * np.sqrt(S[0])
    kr = Vt[0] * np.sqrt(S[0])
    if kc.sum() < 0.0:
        kc, kr = -kc, -kr
    resid = np.abs(np.outer(kc, kr) - k2d).max()
    if resid > 1e-9 * max(1.0, np.abs(k2d).max()):
        raise ValueError(f"blur kernel is not separable (rank-1 resid {resid})")
    return kc, kr


def _band(taps, n=_P):
    """W[j, i] = taps[i + 1 - j]; encodes conv taps + zero padding."""
    taps = np.asarray(taps, dtype=np.float64)
    kh = taps.shape[0]
    W = np.zeros((n, n), dtype=np.float32)
    for b in range(kh):
        off = 1 - b  # input row j = i + 1 - b
        d = np.float32(taps[b])
        idx_i = np.arange(n)
        idx_j = idx_i + off
        m = (idx_j >= 0) & (idx_j < n)
        W[idx_j[m], idx_i[m]] = d
    return W


def _rep(it, repeats):
    for _ in range(repeats):
        yield from it


def _build_program(n_imgs, chunk=_CHUNK, group=_GROUP, repeats=1, wire="bf16",
                   z_copy="scalar", u_copy="vector", parts="full",
                   dma_in="sync", dma_out="sync", xbufs=3, obufs=3,
                   algo="v1", pubufs=3, cgroup=8):
    """Build the per-core Bass program.

    wire: "bf16" (x/u/z all bf16, fp32 PSUM accumulate; x arrives host-
          transposed as [h, n, w] bf16, out leaves as [ho, n, wo] bf16)
          or "fp32" (everything fp32; x as [n, h, w], device-side rearrange).
    z_copy/u_copy: engine for the PSUM->SBUF copies ("vector" or "scalar").
    parts: diagnostic builds — "full", "dma" (transfers only), "compute"
          (DMA only for the first 3 chunks), "mm1" (first-pass matmuls only).
    """
    from contextlib import ExitStack

    import concourse.tile as tile
    from concourse import bacc, mybir

    FP = mybir.dt.float32
    BF = mybir.dt.bfloat16
    DT = BF if wire == "bf16" else FP
    nc = bacc.Bacc("TRN2", target_bir_lowering=False, debug=False)

    if wire == "bf16":
        # host-pretransposed: x[h, n, w], out[ho, n, wo] -> contiguous DMA
        x = nc.declare_dram_parameter("x", [_P, n_imgs, _P], BF, isOutput=False)
        out = nc.declare_dram_parameter("out", [_P, n_imgs, _P], BF, isOutput=True)
        wcr = nc.declare_dram_parameter("wcr", [_P, 2, _P], BF, isOutput=False)
    else:
        x = nc.declare_dram_parameter("x", [n_imgs, _P, _P], FP, isOutput=False)
        out = nc.declare_dram_parameter("out", [n_imgs, _P, _P], FP, isOutput=True)
        wcr = nc.declare_dram_parameter("wcr", [_P, 2, _P], FP, isOutput=False)

    assert n_imgs % chunk == 0 and chunk % group == 0

    with tile.TileContext(nc) as tc, ExitStack() as ctx:
        wp = ctx.enter_context(tc.tile_pool(name="w", bufs=1))
        xp = ctx.enter_context(tc.tile_pool(name="x", bufs=xbufs))
        up = ctx.enter_context(tc.tile_pool(name="u", bufs=2 if algo == "phased" else 4))
        op = ctx.enter_context(tc.tile_pool(name="o", bufs=obufs))
        pzbufs = 2 if (algo == "phased" and cgroup == 8) else 3
        pu = ctx.enter_context(tc.tile_pool(name="pu", bufs=pubufs, space="PSUM"))
        pz = ctx.enter_context(tc.tile_pool(name="pz", bufs=pzbufs, space="PSUM"))

        wt = wp.tile([_P, 2, _P], DT)
        nc.sync.dma_start(wt[:], wcr[:])
        wct = wt[:, 0, :]
        wrt = wt[:, 1, :]

        if not (algo == "phased" and cgroup == 8):
            # dummy matmul absorbs the weight-DMA wait on PE
            scr = pu.tile([_P, 1], FP)
            nc.tensor.matmul(scr[:], lhsT=wct, rhs=wt[:, 0, 0:1],
                             start=True, stop=True)

        engines = {"vector": nc.vector.tensor_copy, "scalar": nc.scalar.copy,
                   "gpsimd": nc.gpsimd.tensor_copy}

        def _rot(spec):
            names = spec.split("+")
            return lambda g: engines[names[g % len(names)]]

        z_eng = _rot(z_copy)
        u_eng = _rot(u_copy)
        dma_engs = {"sync": nc.sync, "scalar": nc.scalar, "vector": nc.vector,
                    "tensor": nc.tensor, "gpsimd": nc.gpsimd}
        din = dma_engs[dma_in]
        dout = dma_engs[dma_out]

        do_compute = parts in ("full", "compute", "mm1")
        do_copies = parts in ("full", "compute")
        xt0 = None
        if parts in ("compute", "mm1"):
            xt0 = xp.tile([_P, chunk, _P], DT)
            if wire == "bf16":
                nc.sync.dma_start(xt0[:], x[:, 0:chunk, :])
            else:
                nc.sync.dma_start(xt0[:], x[0:chunk].rearrange("i h w -> h i w"))
        for cc, c in enumerate(_rep(range(n_imgs // chunk), repeats)):
            if xt0 is not None:
                xt = xt0
            else:
                xt = xp.tile([_P, chunk, _P], DT)
                if wire == "bf16":
                    din.dma_start(xt[:], x[:, c * chunk : (c + 1) * chunk, :])
                else:
                    din.dma_start(
                        xt[:],
                        x[c * chunk : (c + 1) * chunk].rearrange("i h w -> h i w"),
                    )
            ot = op.tile([_P, chunk, _P], DT)
            # 1-element DVE touch: absorbs the out-DMA slot-recycle wait so
            # the first z-copy of the chunk carries only its PE wait.
            nc.vector.memset(ot[:, 0, 0:1], 0.0)
            if algo == "phased" and do_compute:
                # Phase A: data-stationary mm1 for the whole chunk
                #   M_j = x_j^T Wc  -> ust[w, j, ho]  (bf16 SBUF)
                # PSUM tiles hold `cgroup` images (cgroup=8 -> 2 banks) so the
                # PSUM->SBUF copies amortize the fixed PSUM-access latency.
                ust = up.tile([_P, chunk, _P], DT)
                for g in range(chunk // cgroup):
                    put = pu.tile([_P, cgroup, _P], FP)
                    for j in range(cgroup):
                        i = g * cgroup + j
                        nc.tensor.matmul(put[:, j, :], lhsT=xt[:, i, :],
                                         rhs=wct, start=True, stop=True)
                    if do_copies:
                        u_eng(g)(ust[:, g * cgroup : (g + 1) * cgroup, :], put[:])
                # Phase B: weight-stationary mm2, `group` images/instruction
                #   out[wo, (j, ho)] = Wr^T @ [M_0 .. M_3]  ==  z_j^T
                if do_copies:
                    for g in range(chunk // cgroup):
                        pzt = pz.tile([_P, cgroup, _P], FP)
                        for h in range(cgroup // group):
                            lo = h * group
                            nc.tensor.matmul(
                                pzt[:, lo : lo + group, :],
                                lhsT=wrt,
                                rhs=ust[:, g * cgroup + lo : g * cgroup + lo + group, :],
                                start=True,
                                stop=True,
                            )
                        z_eng(g)(ot[:, g * cgroup : (g + 1) * cgroup, :], pzt[:])
            else:
             for g in range(chunk // group):
                if not do_compute:
                    continue
                put = pu.tile([_P, group, _P], FP)
                for j in range(group):
                    i = g * group + j
                    nc.tensor.matmul(put[:, j, :], lhsT=xt[:, i, :],
                                     rhs=wct, start=True, stop=True)
                if not do_copies:
                    continue
                ust = up.tile([_P, group, _P], DT)
                u_eng(g)(ust[:], put[:])
                pzt = pz.tile([_P, group, _P], FP)
                for j in range(group):
                    nc.tensor.matmul(
                        pzt[:, j, :],
                        lhsT=ust[:, j, :],
                        rhs=wrt,
                        start=True,
                        stop=True,
                    )
                z_eng(g)(ot[:, g * group : (g + 1) * group, :], pzt[:])
            if parts in ("full", "dma"):
                if wire == "bf16":
                    dout.dma_start(out[:, c * chunk : (c + 1) * chunk, :], ot[:])
                else:
                    dout.dma_start(
                        out[c * chunk : (c + 1) * chunk].rearrange("i h w -> h i w"),
                        ot[:],
                    )

    nc.compile()
    return nc


def _build_kwargs(cfg):
    return dict(chunk=cfg["chunk"], wire=cfg["wire"], z_copy=cfg["z_copy"],
                u_copy=cfg["u_copy"], dma_in=cfg["dma_in"], dma_out=cfg["dma_out"],
                xbufs=cfg["xbufs"], obufs=cfg["obufs"], algo=cfg["algo"],
                pubufs=cfg["pubufs"], cgroup=cfg["cgroup"])


def _get_program(n_imgs, cfg):
    key = (n_imgs,) + tuple(sorted(_build_kwargs(cfg).items()))
    if key not in _PROGRAM_CACHE:
        _PROGRAM_CACHE[key] = _build_program(n_imgs, **_build_kwargs(cfg))
    return _PROGRAM_CACHE[key]


def _prepare_in_maps(input, kernel, cfg):
    """Shard + pack FULL inputs into per-core in_maps for the Bass program."""
    import ml_dtypes

    x = np.asarray(input, dtype=np.float32)
    imgs = x.reshape(-1, _P, _P)
    n = imgs.shape[0]
    assert n % _NCORES == 0
    per = n // _NCORES

    kc, kr = _taps_from_kernel2d(kernel)
    Wc = _band(kc)
    Wr = _band(kr)

    if cfg["wire"] == "bf16":
        bf = ml_dtypes.bfloat16
        wcr_f = np.stack([Wc, Wr], axis=1)  # [j, 2, i]
        if not np.all(wcr_f.astype(bf).astype(np.float32) == wcr_f):
            raise ValueError("blur taps not exact in bf16")
        wcr = np.ascontiguousarray(wcr_f.astype(bf))
        xs = imgs.astype(bf)
        in_maps = [
            {
                "x": np.ascontiguousarray(
                    xs[i * per : (i + 1) * per].transpose(1, 0, 2)
                ),
                "wcr": wcr,
            }
            for i in range(_NCORES)
        ]
    else:
        wcr = np.ascontiguousarray(np.stack([Wc, Wr], axis=1))
        in_maps = [
            {"x": np.ascontiguousarray(imgs[i * per : (i + 1) * per]), "wcr": wcr}
            for i in range(_NCORES)
        ]
    return in_maps, per


def _assemble_output(outs, orig_shape, cfg):
    """Per-core device outputs -> FULL fp32 output of the original shape."""
    if cfg["wire"] == "bf16":
        if cfg["algo"] == "phased":
            # each out is [wo, per, ho] bf16 (z^T on the wire)
            full = np.concatenate(
                [np.asarray(o).transpose(1, 2, 0) for o in outs], axis=0
            ).astype(np.float32)
        else:
            # each out is [ho, per, wo] bf16
            full = np.concatenate(
                [np.asarray(o).transpose(1, 0, 2) for o in outs], axis=0
            ).astype(np.float32)
    else:
        full = np.concatenate([np.asarray(o) for o in outs], axis=0)
    return np.ascontiguousarray(full).reshape(orig_shape)


def kernel(input, kernel, _trace=False, _cfg=None):
    global LAST_RESULTS
    from concourse.bass_utils import run_bass_kernel_spmd

    cfg = dict(_CFG)
    if _cfg:
        cfg.update(_cfg)

    orig_shape = np.asarray(input).shape
    in_maps, per = _prepare_in_maps(input, kernel, cfg)
    nc = _get_program(per, cfg)
    res = run_bass_kernel_spmd(
        nc, in_maps, core_ids=list(range(_NCORES)), trace=_trace
    )
    LAST_RESULTS = res
    outs = [res.results[i]["out"] for i in range(_NCORES)]
    return _assemble_output(outs, orig_shape, cfg)


# revision 18
# speedup vs baseline: 1.1872x; 1.0551x over previous
"""Trainium2 Bass kernel for nn_Blur3d (4x4 separable blur, pad=(2,1)).

Math: reference does a true 2D convolution of each (h, w) = (128, 128) slice
with the normalized 4x4 blur kernel K2 (an outer product of a 1D 4-tap
kernel), zero-padded by (2, 1) on each spatial dim.  For output index i:
    out[i, j] = sum_{bh, bw} K2[bh, bw] * x[i + 1 - bh, j + 1 - bw]
With K2 = outer(kc, kr) this is z = Wc^T @ x @ Wr where Wc/Wr are 128x128
banded matrices: W[j, i] = k[i + 1 - j] (0 <= i + 1 - j < 4).

On-device, default algo="phased" (per 64-image chunk, x as [h=128 part,
w=128 free]):
    Phase A (data-stationary): M_j = matmul(lhsT=x_j, rhs=Wc) = x_j^T Wc
        -> PSUM [w part, ho free], drained to SBUF as bf16.
    Phase B (weight-stationary): matmul(lhsT=Wr, rhs=[M_0..M_3])
        = Wr^T [M_0..M_3] = [z_0^T..z_3^T], 4 images per instruction with
        the stationary operand (Wr) loaded once -> no per-image LDWEIGHTS.
The output leaves the device as z^T; the host un-transposes during its
re-layout pass.  The banded matrices encode the zero padding exactly.

Wire format (default "bf16"): x is cast to bf16 on host and shipped in a
pre-transposed [h, n, w] layout so every DMA is 128 partitions x 16 KiB
contiguous; the band-matrix taps are exactly representable in bf16; both
matmuls run at full bf16 PE rate with fp32 PSUM accumulation.  PSUM->SBUF
drains copy 8 images per instruction (2 PSUM banks) to amortize the fixed
PSUM-access latency and alternate between the DVE and Scalar engines to
stay off the critical path.  The host upcasts the bf16 result to fp32.
End-to-end max rel err vs the fp32 reference is 5.3e-3 (quantization of
x, u, z at 8 mantissa bits each), while HBM traffic is halved and PE time
is ~4x lower vs the fp32 path.

Sharding: 4096 independent images, 512 per core, pure data parallel on the
collapsed (batch, c, t) dim across 8 NeuronCores.
"""

import numpy as np

_P = 128          # image height/width and partition count
_NCORES = 8
_CHUNK = 32       # images per DMA chunk
_GROUP = 4        # images per PSUM bank (4 * 128 fp32 = one 2 KiB bank)

# Default build configuration (can be overridden for experiments).
_CFG = dict(wire="bf16", z_copy="scalar+vector", u_copy="vector+scalar",
            chunk=64, dma_in="sync", dma_out="sync", xbufs=3, obufs=3,
            algo="phased", pubufs=2, cgroup=8, ilv=0, osplit=1, pfetch=2)

_PROGRAM_CACHE = {}
LAST_RESULTS = None  # BassKernelResults of the most recent run (for profiling)


def _taps_from_kernel2d(k2d):
    """Rank-1 (separable) decomposition of the 4x4 blur kernel."""
    k2d = np.asarray(k2d, dtype=np.float64)
    U, S, Vt = np.linalg.svd(k2d)
    kc = U[:, 0] * np.sqrt(S[0])
    kr = Vt[0] * np.sqrt(S[0])
    if kc.sum() < 0.0:
        kc, kr = -kc, -kr
    resid = np.abs(np.outer(kc, kr) - k2d).max()
    if resid > 1e-9 * max(1.0, np.abs(k2d).max()):
        raise ValueError(f"blur kernel is not separable (rank-1 resid {resid})")
    return kc, kr


def _band(taps, n=_P):
    """W[j, i] = taps[i + 1 - j]; encodes conv taps + zero padding."""
    taps = np.asarray(taps, dtype=np.float64)
    kh = taps.shape[0]
    W = np.zeros((n, n), dtype=np.float32)
    for b in range(kh):
        off = 1 - b  # input row j = i + 1 - b
        d = np.float32(taps[b])
        idx_i = np.arange(n)
        idx_j = idx_i + off
        m = (idx_j >= 0) & (idx_j < n)
        W[idx_j[m], idx_i[m]] = d
    return W


def _rep(it, repeats):
    for _ in range(repeats):
        yield from it


def _build_program(n_imgs, chunk=_CHUNK, group=_GROUP, repeats=1, wire="bf16",
                   z_copy="scalar", u_copy="vector", parts="full",
                   dma_in="sync", dma_out="sync", xbufs=3, obufs=3,
                   algo="v1", pubufs=3, cgroup=8, ilv=0, osplit=1, pfetch=0):
    """Build the per-core Bass program.

    wire: "bf16" (x/u/z all bf16, fp32 PSUM accumulate; x arrives host-
          transposed as [h, n, w] bf16, out leaves as [ho, n, wo] bf16)
          or "fp32" (everything fp32; x as [n, h, w], device-side rearrange).
    z_copy/u_copy: engine for the PSUM->SBUF copies ("vector" or "scalar").
    parts: diagnostic builds — "full", "dma" (transfers only), "compute"
          (DMA only for the first 3 chunks), "mm1" (first-pass matmuls only).
    """
    from contextlib import ExitStack

    import concourse.tile as tile
    from concourse import bacc, mybir

    FP = mybir.dt.float32
    BF = mybir.dt.bfloat16
    DT = BF if wire == "bf16" else FP
    nc = bacc.Bacc("TRN2", target_bir_lowering=False, debug=False)

    if wire == "bf16":
        # host-pretransposed: x[h, n, w], out[ho, n, wo] -> contiguous DMA
        x = nc.declare_dram_parameter("x", [_P, n_imgs, _P], BF, isOutput=False)
        out = nc.declare_dram_parameter("out", [_P, n_imgs, _P], BF, isOutput=True)
        wcr = nc.declare_dram_parameter("wcr", [_P, 2, _P], BF, isOutput=False)
    else:
        x = nc.declare_dram_parameter("x", [n_imgs, _P, _P], FP, isOutput=False)
        out = nc.declare_dram_parameter("out", [n_imgs, _P, _P], FP, isOutput=True)
        wcr = nc.declare_dram_parameter("wcr", [_P, 2, _P], FP, isOutput=False)

    assert n_imgs % chunk == 0 and chunk % group == 0

    with tile.TileContext(nc) as tc, ExitStack() as ctx:
        wp = ctx.enter_context(tc.tile_pool(name="w", bufs=1))
        xp = ctx.enter_context(tc.tile_pool(name="x", bufs=xbufs))
        up = ctx.enter_context(tc.tile_pool(name="u", bufs=2 if algo == "phased" else 4))
        op = ctx.enter_context(tc.tile_pool(name="o", bufs=obufs))
        pzbufs = 2 if (algo == "phased" and cgroup == 8) else 3
        pu = ctx.enter_context(tc.tile_pool(name="pu", bufs=pubufs, space="PSUM"))
        pz = ctx.enter_context(tc.tile_pool(name="pz", bufs=pzbufs, space="PSUM"))

        wt = wp.tile([_P, 2, _P], DT)
        nc.sync.dma_start(wt[:], wcr[:])
        wct = wt[:, 0, :]
        wrt = wt[:, 1, :]

        if not (algo == "phased" and cgroup == 8):
            # dummy matmul absorbs the weight-DMA wait on PE
            scr = pu.tile([_P, 1], FP)
            nc.tensor.matmul(scr[:], lhsT=wct, rhs=wt[:, 0, 0:1],
                             start=True, stop=True)

        engines = {"vector": nc.vector.tensor_copy, "scalar": nc.scalar.copy,
                   "gpsimd": nc.gpsimd.tensor_copy}

        def _rot(spec):
            names = spec.split("+")
            return lambda g: engines[names[g % len(names)]]

        z_eng = _rot(z_copy)
        u_eng = _rot(u_copy)
        dma_engs = {"sync": nc.sync, "scalar": nc.scalar, "vector": nc.vector,
                    "tensor": nc.tensor, "gpsimd": nc.gpsimd}
        din = dma_engs[dma_in]
        dout = dma_engs[dma_out]

        do_compute = parts in ("full", "compute", "mm1")
        do_copies = parts in ("full", "compute")
        xt0 = None
        if parts in ("compute", "mm1"):
            xt0 = xp.tile([_P, chunk, _P], DT)
            if wire == "bf16":
                nc.sync.dma_start(xt0[:], x[:, 0:chunk, :])
            else:
                nc.sync.dma_start(xt0[:], x[0:chunk].rearrange("i h w -> h i w"))

        # Software prefetch: issue in-DMAs `pfetch` chunks ahead so they sit
        # in the issuing sequencer's stream BEFORE the out-DMA of the current
        # chunk.  The out-DMA blocks at the sequencer until the chunk's
        # z-copies complete; without prefetch that head-of-line-blocks the
        # next chunk's input load and serializes DMA with compute.
        seq = list(_rep(range(n_imgs // chunk), repeats))
        from collections import deque
        pending = deque()

        def issue_in(idx):
            xt = xp.tile([_P, chunk, _P], DT)
            ci = seq[idx]
            if wire == "bf16":
                din.dma_start(xt[:], x[:, ci * chunk : (ci + 1) * chunk, :])
            else:
                din.dma_start(
                    xt[:],
                    x[ci * chunk : (ci + 1) * chunk].rearrange("i h w -> h i w"),
                )
            pending.append(xt)

        if xt0 is None:
            for k in range(min(pfetch, len(seq))):
                issue_in(k)
        for cc, c in enumerate(seq):
            if xt0 is not None:
                xt = xt0
            else:
                if cc + pfetch < len(seq):
                    issue_in(cc + pfetch)
                if pending:
                    xt = pending.popleft()
                else:
                    issue_in(cc)
                    xt = pending.popleft()
            ot = op.tile([_P, chunk, _P], DT)
            # 1-element DVE touch: absorbs the out-DMA slot-recycle wait so
            # the first z-copy of the chunk carries only its PE wait.
            nc.vector.memset(ot[:, 0, 0:1], 0.0)
            if algo == "phased" and do_compute:
                # Phase A: data-stationary mm1 for the whole chunk
                #   M_j = x_j^T Wc  -> ust[w, j, ho]  (bf16 SBUF)
                # PSUM tiles hold `cgroup` images (cgroup=8 -> 2 banks) so the
                # PSUM->SBUF copies amortize the fixed PSUM-access latency.
                # Phase B (weight-stationary mm2, z_j^T = Wr^T M_j, `group`
                # images/instruction) is software-pipelined `ilv` tiles behind
                # phase A so PE never stalls on a u-drain and z-copies spread
                # across the chunk instead of bunching at its end.
                ust = up.tile([_P, chunk, _P], DT)
                ntiles = chunk // cgroup
                lag = ilv if do_copies else 0

                def phase_b(gb):
                    pzt = pz.tile([_P, cgroup, _P], FP)
                    for h in range(cgroup // group):
                        lo = h * group
                        nc.tensor.matmul(
                            pzt[:, lo : lo + group, :],
                            lhsT=wrt,
                            rhs=ust[:, gb * cgroup + lo : gb * cgroup + lo + group, :],
                            start=True,
                            stop=True,
                        )
                    z_eng(gb)(ot[:, gb * cgroup : (gb + 1) * cgroup, :], pzt[:])
                    if osplit == 2 and gb == ntiles // 2 - 1:
                        dout.dma_start(
                            out[:, c * chunk : c * chunk + chunk // 2, :],
                            ot[:, : chunk // 2, :],
                        )

                for g in range(ntiles + lag):
                    if g < ntiles:
                        put = pu.tile([_P, cgroup, _P], FP)
                        for j in range(cgroup):
                            i = g * cgroup + j
                            nc.tensor.matmul(put[:, j, :], lhsT=xt[:, i, :],
                                             rhs=wct, start=True, stop=True)
                        if do_copies:
                            u_eng(g)(ust[:, g * cgroup : (g + 1) * cgroup, :],
                                     put[:])
                    if do_copies and lag > 0 and 0 <= g - lag < ntiles:
                        phase_b(g - lag)
                if do_copies and lag == 0:
                    for gb in range(ntiles):
                        phase_b(gb)
            else:
             for g in range(chunk // group):
                if not do_compute:
                    continue
                put = pu.tile([_P, group, _P], FP)
                for j in range(group):
                    i = g * group + j
                    nc.tensor.matmul(put[:, j, :], lhsT=xt[:, i, :],
                                     rhs=wct, start=True, stop=True)
                if not do_copies:
                    continue
                ust = up.tile([_P, group, _P], DT)
                u_eng(g)(ust[:], put[:])
                pzt = pz.tile([_P, group, _P], FP)
                for j in range(group):
                    nc.tensor.matmul(
                        pzt[:, j, :],
                        lhsT=ust[:, j, :],
                        rhs=wrt,
                        start=True,
                        stop=True,
                    )
                z_eng(g)(ot[:, g * group : (g + 1) * group, :], pzt[:])
            if parts in ("full", "dma"):
                if wire == "bf16":
                    if osplit == 2 and parts == "full" and algo == "phased":
                        dout.dma_start(
                            out[:, c * chunk + chunk // 2 : (c + 1) * chunk, :],
                            ot[:, chunk // 2 :, :],
                        )
                    else:
                        dout.dma_start(out[:, c * chunk : (c + 1) * chunk, :], ot[:])
                else:
                    dout.dma_start(
                        out[c * chunk : (c + 1) * chunk].rearrange("i h w -> h i w"),
                        ot[:],
                    )

    nc.compile()
    return nc


def _build_kwargs(cfg):
    return dict(chunk=cfg["chunk"], wire=cfg["wire"], z_copy=cfg["z_copy"],
                u_copy=cfg["u_copy"], dma_in=cfg["dma_in"], dma_out=cfg["dma_out"],
                xbufs=cfg["xbufs"], obufs=cfg["obufs"], algo=cfg["algo"],
                pubufs=cfg["pubufs"], cgroup=cfg["cgroup"], ilv=cfg["ilv"],
                osplit=cfg["osplit"], pfetch=cfg["pfetch"])


def _get_program(n_imgs, cfg):
    key = (n_imgs,) + tuple(sorted(_build_kwargs(cfg).items()))
    if key not in _PROGRAM_CACHE:
        _PROGRAM_CACHE[key] = _build_program(n_imgs, **_build_kwargs(cfg))
    return _PROGRAM_CACHE[key]


def _prepare_in_maps(input, kernel, cfg):
    """Shard + pack FULL inputs into per-core in_maps for the Bass program."""
    import ml_dtypes

    x = np.asarray(input, dtype=np.float32)
    imgs = x.reshape(-1, _P, _P)
    n = imgs.shape[0]
    assert n % _NCORES == 0
    per = n // _NCORES

    kc, kr = _taps_from_kernel2d(kernel)
    Wc = _band(kc)
    Wr = _band(kr)

    if cfg["wire"] == "bf16":
        bf = ml_dtypes.bfloat16
        wcr_f = np.stack([Wc, Wr], axis=1)  # [j, 2, i]
        if not np.all(wcr_f.astype(bf).astype(np.float32) == wcr_f):
            raise ValueError("blur taps not exact in bf16")
        wcr = np.ascontiguousarray(wcr_f.astype(bf))
        xs = imgs.astype(bf)
        in_maps = [
            {
                "x": np.ascontiguousarray(
                    xs[i * per : (i + 1) * per].transpose(1, 0, 2)
                ),
                "wcr": wcr,
            }
            for i in range(_NCORES)
        ]
    else:
        wcr = np.ascontiguousarray(np.stack([Wc, Wr], axis=1))
        in_maps = [
            {"x": np.ascontiguousarray(imgs[i * per : (i + 1) * per]), "wcr": wcr}
            for i in range(_NCORES)
        ]
    return in_maps, per


def _assemble_output(outs, orig_shape, cfg):
    """Per-core device outputs -> FULL fp32 output of the original shape."""
    if cfg["wire"] == "bf16":
        if cfg["algo"] == "phased":
            # each out is [wo, per, ho] bf16 (z^T on the wire)
            full = np.concatenate(
                [np.asarray(o).transpose(1, 2, 0) for o in outs], axis=0
            ).astype(np.float32)
        else:
            # each out is [ho, per, wo] bf16
            full = np.concatenate(
                [np.asarray(o).transpose(1, 0, 2) for o in outs], axis=0
            ).astype(np.float32)
    else:
        full = np.concatenate([np.asarray(o) for o in outs], axis=0)
    return np.ascontiguousarray(full).reshape(orig_shape)


def kernel(input, kernel, _trace=False, _cfg=None):
    global LAST_RESULTS
    from concourse.bass_utils import run_bass_kernel_spmd

    cfg = dict(_CFG)
    if _cfg:
        cfg.update(_cfg)

    orig_shape = np.asarray(input).shape
    in_maps, per = _prepare_in_maps(input, kernel, cfg)
    nc = _get_program(per, cfg)
    res = run_bass_kernel_spmd(
        nc, in_maps, core_ids=list(range(_NCORES)), trace=_trace
    )
    LAST_RESULTS = res
    outs = [res.results[i]["out"] for i in range(_NCORES)]
    return _assemble_output(outs, orig_shape, cfg)


# revision 19
# speedup vs baseline: 1.6261x; 1.3697x over previous
"""Trainium2 Bass kernel for nn_Blur3d (4x4 separable blur, pad=(2,1)).

Math: reference does a true 2D convolution of each (h, w) = (128, 128) slice
with the normalized 4x4 blur kernel K2 (an outer product of a 1D 4-tap
kernel), zero-padded by (2, 1) on each spatial dim.  For output index i:
    out[i, j] = sum_{bh, bw} K2[bh, bw] * x[i + 1 - bh, j + 1 - bw]
With K2 = outer(kc, kr) this is z = Wc^T @ x @ Wr where Wc/Wr are 128x128
banded matrices: W[j, i] = k[i + 1 - j] (0 <= i + 1 - j < 4).

On-device, default algo="phased" (per 64-image chunk, x as [h=128 part,
w=128 free]):
    Phase A (data-stationary): M_j = matmul(lhsT=x_j, rhs=Wc) = x_j^T Wc
        -> PSUM [w part, ho free], drained to SBUF as bf16.
    Phase B (weight-stationary): matmul(lhsT=Wr, rhs=[M_0..M_3])
        = Wr^T [M_0..M_3] = [z_0^T..z_3^T], 4 images per instruction with
        the stationary operand (Wr) loaded once -> no per-image LDWEIGHTS.
The output leaves the device as z^T; the host un-transposes during its
re-layout pass.  The banded matrices encode the zero padding exactly.

Wire format (default "bf16"): x is cast to bf16 on host and shipped in a
pre-transposed [h, n, w] layout so every DMA is 128 partitions x 16 KiB
contiguous; the band-matrix taps are exactly representable in bf16; both
matmuls run at full bf16 PE rate with fp32 PSUM accumulation.  PSUM->SBUF
drains copy 8 images per instruction (2 PSUM banks) to amortize the fixed
PSUM-access latency and alternate between the DVE and Scalar engines to
stay off the critical path.  Input DMAs are issued two chunks ahead so the
out-DMA (which blocks at the sequencer until the chunk's z-drains finish)
does not head-of-line-block input prefetch.  The host upcasts the bf16
result to fp32.
End-to-end max rel err vs the fp32 reference is 5.3e-3 (quantization of
x, u, z at 8 mantissa bits each), while HBM traffic is halved and PE time
is ~4x lower vs the fp32 path.

Sharding: 4096 independent images, 512 per core, pure data parallel on the
collapsed (batch, c, t) dim across 8 NeuronCores.
"""

import numpy as np

_P = 128          # image height/width and partition count
_NCORES = 8
_CHUNK = 32       # images per DMA chunk
_GROUP = 4        # images per PSUM bank (4 * 128 fp32 = one 2 KiB bank)

# Default build configuration (can be overridden for experiments).
_CFG = dict(wire="bf16", z_copy="scalar+vector", u_copy="vector+scalar",
            chunk=64, dma_in="sync", dma_out="sync", xbufs=3, obufs=3,
            algo="phased", pubufs=2, cgroup=8, ilv=0, osplit=1, pfetch=2)

_PROGRAM_CACHE = {}
LAST_RESULTS = None  # BassKernelResults of the most recent run (for profiling)


def _taps_from_kernel2d(k2d):
    """Rank-1 (separable) decomposition of the 4x4 blur kernel."""
    k2d = np.asarray(k2d, dtype=np.float64)
    U, S, Vt = np.linalg.svd(k2d)
    kc = U[:, 0] * np.sqrt(S[0])
    kr = Vt[0] * np.sqrt(S[0])
    if kc.sum() < 0.0:
        kc, kr = -kc, -kr
    resid = np.abs(np.outer(kc, kr) - k2d).max()
    if resid > 1e-9 * max(1.0, np.abs(k2d).max()):
        raise ValueError(f"blur kernel is not separable (rank-1 resid {resid})")
    return kc, kr


def _band(taps, n=_P):
    """W[j, i] = taps[i + 1 - j]; encodes conv taps + zero padding."""
    taps = np.asarray(taps, dtype=np.float64)
    kh = taps.shape[0]
    W = np.zeros((n, n), dtype=np.float32)
    for b in range(kh):
        off = 1 - b  # input row j = i + 1 - b
        d = np.float32(taps[b])
        idx_i = np.arange(n)
        idx_j = idx_i + off
        m = (idx_j >= 0) & (idx_j < n)
        W[idx_j[m], idx_i[m]] = d
    return W


def _rep(it, repeats):
    for _ in range(repeats):
        yield from it


def _build_program(n_imgs, chunk=_CHUNK, group=_GROUP, repeats=1, wire="bf16",
                   z_copy="scalar", u_copy="vector", parts="full",
                   dma_in="sync", dma_out="sync", xbufs=3, obufs=3,
                   algo="v1", pubufs=3, cgroup=8, ilv=0, osplit=1, pfetch=0):
    """Build the per-core Bass program.

    wire: "bf16" (x/u/z all bf16, fp32 PSUM accumulate; x arrives host-
          transposed as [h, n, w] bf16, out leaves as [ho, n, wo] bf16)
          or "fp32" (everything fp32; x as [n, h, w], device-side rearrange).
    z_copy/u_copy: engine for the PSUM->SBUF copies ("vector" or "scalar").
    parts: diagnostic builds — "full", "dma" (transfers only), "compute"
          (DMA only for the first 3 chunks), "mm1" (first-pass matmuls only).
    """
    from contextlib import ExitStack

    import concourse.tile as tile
    from concourse import bacc, mybir

    FP = mybir.dt.float32
    BF = mybir.dt.bfloat16
    DT = BF if wire == "bf16" else FP
    nc = bacc.Bacc("TRN2", target_bir_lowering=False, debug=False)

    if wire == "bf16":
        # host-pretransposed: x[h, n, w], out[ho, n, wo] -> contiguous DMA
        x = nc.declare_dram_parameter("x", [_P, n_imgs, _P], BF, isOutput=False)
        out = nc.declare_dram_parameter("out", [_P, n_imgs, _P], BF, isOutput=True)
        wcr = nc.declare_dram_parameter("wcr", [_P, 2, _P], BF, isOutput=False)
    else:
        x = nc.declare_dram_parameter("x", [n_imgs, _P, _P], FP, isOutput=False)
        out = nc.declare_dram_parameter("out", [n_imgs, _P, _P], FP, isOutput=True)
        wcr = nc.declare_dram_parameter("wcr", [_P, 2, _P], FP, isOutput=False)

    assert n_imgs % chunk == 0 and chunk % group == 0

    with tile.TileContext(nc) as tc, ExitStack() as ctx:
        wp = ctx.enter_context(tc.tile_pool(name="w", bufs=1))
        xp = ctx.enter_context(tc.tile_pool(name="x", bufs=xbufs))
        up = ctx.enter_context(tc.tile_pool(name="u", bufs=2 if algo == "phased" else 4))
        op = ctx.enter_context(tc.tile_pool(name="o", bufs=obufs))
        pzbufs = 2 if (algo == "phased" and cgroup == 8) else 3
        pu = ctx.enter_context(tc.tile_pool(name="pu", bufs=pubufs, space="PSUM"))
        pz = ctx.enter_context(tc.tile_pool(name="pz", bufs=pzbufs, space="PSUM"))

        wt = wp.tile([_P, 2, _P], DT)
        nc.sync.dma_start(wt[:], wcr[:])
        wct = wt[:, 0, :]
        wrt = wt[:, 1, :]

        if not (algo == "phased" and cgroup == 8):
            # dummy matmul absorbs the weight-DMA wait on PE
            scr = pu.tile([_P, 1], FP)
            nc.tensor.matmul(scr[:], lhsT=wct, rhs=wt[:, 0, 0:1],
                             start=True, stop=True)

        engines = {"vector": nc.vector.tensor_copy, "scalar": nc.scalar.copy,
                   "gpsimd": nc.gpsimd.tensor_copy}

        def _rot(spec):
            names = spec.split("+")
            return lambda g: engines[names[g % len(names)]]

        z_eng = _rot(z_copy)
        u_eng = _rot(u_copy)
        dma_engs = {"sync": nc.sync, "scalar": nc.scalar, "vector": nc.vector,
                    "tensor": nc.tensor, "gpsimd": nc.gpsimd}
        din = dma_engs[dma_in]
        dout = dma_engs[dma_out]

        do_compute = parts in ("full", "compute", "mm1")
        do_copies = parts in ("full", "compute")
        xt0 = None
        if parts in ("compute", "mm1"):
            xt0 = xp.tile([_P, chunk, _P], DT)
            if wire == "bf16":
                nc.sync.dma_start(xt0[:], x[:, 0:chunk, :])
            else:
                nc.sync.dma_start(xt0[:], x[0:chunk].rearrange("i h w -> h i w"))

        # Software prefetch: issue in-DMAs `pfetch` chunks ahead so they sit
        # in the issuing sequencer's stream BEFORE the out-DMA of the current
        # chunk.  The out-DMA blocks at the sequencer until the chunk's
        # z-copies complete; without prefetch that head-of-line-blocks the
        # next chunk's input load and serializes DMA with compute.
        seq = list(_rep(range(n_imgs // chunk), repeats))
        from collections import deque
        pending = deque()

        def issue_in(idx):
            xt = xp.tile([_P, chunk, _P], DT)
            ci = seq[idx]
            if wire == "bf16":
                din.dma_start(xt[:], x[:, ci * chunk : (ci + 1) * chunk, :])
            else:
                din.dma_start(
                    xt[:],
                    x[ci * chunk : (ci + 1) * chunk].rearrange("i h w -> h i w"),
                )
            pending.append(xt)

        if xt0 is None:
            for k in range(min(pfetch, len(seq))):
                issue_in(k)
        for cc, c in enumerate(seq):
            if xt0 is not None:
                xt = xt0
            else:
                if cc + pfetch < len(seq):
                    issue_in(cc + pfetch)
                if pending:
                    xt = pending.popleft()
                else:
                    issue_in(cc)
                    xt = pending.popleft()
            ot = op.tile([_P, chunk, _P], DT)
            # 1-element DVE touch: absorbs the out-DMA slot-recycle wait so
            # the first z-copy of the chunk carries only its PE wait.
            nc.vector.memset(ot[:, 0, 0:1], 0.0)
            if algo == "phased" and do_compute:
                # Phase A: data-stationary mm1 for the whole chunk
                #   M_j = x_j^T Wc  -> ust[w, j, ho]  (bf16 SBUF)
                # PSUM tiles hold `cgroup` images (cgroup=8 -> 2 banks) so the
                # PSUM->SBUF copies amortize the fixed PSUM-access latency.
                # Phase B (weight-stationary mm2, z_j^T = Wr^T M_j, `group`
                # images/instruction) is software-pipelined `ilv` tiles behind
                # phase A so PE never stalls on a u-drain and z-copies spread
                # across the chunk instead of bunching at its end.
                ust = up.tile([_P, chunk, _P], DT)
                ntiles = chunk // cgroup
                lag = ilv if do_copies else 0

                def phase_b(gb):
                    pzt = pz.tile([_P, cgroup, _P], FP)
                    for h in range(cgroup // group):
                        lo = h * group
                        nc.tensor.matmul(
                            pzt[:, lo : lo + group, :],
                            lhsT=wrt,
                            rhs=ust[:, gb * cgroup + lo : gb * cgroup + lo + group, :],
                            start=True,
                            stop=True,
                        )
                    z_eng(gb)(ot[:, gb * cgroup : (gb + 1) * cgroup, :], pzt[:])
                    if osplit == 2 and gb == ntiles // 2 - 1:
                        dout.dma_start(
                            out[:, c * chunk : c * chunk + chunk // 2, :],
                            ot[:, : chunk // 2, :],
                        )

                for g in range(ntiles + lag):
                    if g < ntiles:
                        put = pu.tile([_P, cgroup, _P], FP)
                        for j in range(cgroup):
                            i = g * cgroup + j
                            nc.tensor.matmul(put[:, j, :], lhsT=xt[:, i, :],
                                             rhs=wct, start=True, stop=True)
                        if do_copies:
                            u_eng(g)(ust[:, g * cgroup : (g + 1) * cgroup, :],
                                     put[:])
                    if do_copies and lag > 0 and 0 <= g - lag < ntiles:
                        phase_b(g - lag)
                if do_copies and lag == 0:
                    for gb in range(ntiles):
                        phase_b(gb)
            else:
             for g in range(chunk // group):
                if not do_compute:
                    continue
                put = pu.tile([_P, group, _P], FP)
                for j in range(group):
                    i = g * group + j
                    nc.tensor.matmul(put[:, j, :], lhsT=xt[:, i, :],
                                     rhs=wct, start=True, stop=True)
                if not do_copies:
                    continue
                ust = up.tile([_P, group, _P], DT)
                u_eng(g)(ust[:], put[:])
                pzt = pz.tile([_P, group, _P], FP)
                for j in range(group):
                    nc.tensor.matmul(
                        pzt[:, j, :],
                        lhsT=ust[:, j, :],
                        rhs=wrt,
                        start=True,
                        stop=True,
                    )
                z_eng(g)(ot[:, g * group : (g + 1) * group, :], pzt[:])
            if parts in ("full", "dma"):
                if wire == "bf16":
                    if osplit == 2 and parts == "full" and algo == "phased":
                        dout.dma_start(
                            out[:, c * chunk + chunk // 2 : (c + 1) * chunk, :],
                            ot[:, chunk // 2 :, :],
                        )
                    else:
                        dout.dma_start(out[:, c * chunk : (c + 1) * chunk, :], ot[:])
                else:
                    dout.dma_start(
                        out[c * chunk : (c + 1) * chunk].rearrange("i h w -> h i w"),
                        ot[:],
                    )

    nc.compile()
    return nc


def _build_kwargs(cfg):
    return dict(chunk=cfg["chunk"], wire=cfg["wire"], z_copy=cfg["z_copy"],
                u_copy=cfg["u_copy"], dma_in=cfg["dma_in"], dma_out=cfg["dma_out"],
                xbufs=cfg["xbufs"], obufs=cfg["obufs"], algo=cfg["algo"],
                pubufs=cfg["pubufs"], cgroup=cfg["cgroup"], ilv=cfg["ilv"],
                osplit=cfg["osplit"], pfetch=cfg["pfetch"])


def _get_program(n_imgs, cfg):
    key = (n_imgs,) + tuple(sorted(_build_kwargs(cfg).items()))
    if key not in _PROGRAM_CACHE:
        _PROGRAM_CACHE[key] = _build_program(n_imgs, **_build_kwargs(cfg))
    return _PROGRAM_CACHE[key]


def _prepare_in_maps(input, kernel, cfg):
    """Shard + pack FULL inputs into per-core in_maps for the Bass program."""
    import ml_dtypes

    x = np.asarray(input, dtype=np.float32)
    imgs = x.reshape(-1, _P, _P)
    n = imgs.shape[0]
    assert n % _NCORES == 0
    per = n // _NCORES

    kc, kr = _taps_from_kernel2d(kernel)
    Wc = _band(kc)
    Wr = _band(kr)

    if cfg["wire"] == "bf16":
        bf = ml_dtypes.bfloat16
        wcr_f = np.stack([Wc, Wr], axis=1)  # [j, 2, i]
        if not np.all(wcr_f.astype(bf).astype(np.float32) == wcr_f):
            raise ValueError("blur taps not exact in bf16")
        wcr = np.ascontiguousarray(wcr_f.astype(bf))
        xs = imgs.astype(bf)
        in_maps = [
            {
                "x": np.ascontiguousarray(
                    xs[i * per : (i + 1) * per].transpose(1, 0, 2)
                ),
                "wcr": wcr,
            }
            for i in range(_NCORES)
        ]
    else:
        wcr = np.ascontiguousarray(np.stack([Wc, Wr], axis=1))
        in_maps = [
            {"x": np.ascontiguousarray(imgs[i * per : (i + 1) * per]), "wcr": wcr}
            for i in range(_NCORES)
        ]
    return in_maps, per


def _assemble_output(outs, orig_shape, cfg):
    """Per-core device outputs -> FULL fp32 output of the original shape."""
    if cfg["wire"] == "bf16":
        if cfg["algo"] == "phased":
            # each out is [wo, per, ho] bf16 (z^T on the wire)
            full = np.concatenate(
                [np.asarray(o).transpose(1, 2, 0) for o in outs], axis=0
            ).astype(np.float32)
        else:
            # each out is [ho, per, wo] bf16
            full = np.concatenate(
                [np.asarray(o).transpose(1, 0, 2) for o in outs], axis=0
            ).astype(np.float32)
    else:
        full = np.concatenate([np.asarray(o) for o in outs], axis=0)
    return np.ascontiguousarray(full).reshape(orig_shape)


def kernel(input, kernel, _trace=False, _cfg=None):
    global LAST_RESULTS
    from concourse.bass_utils import run_bass_kernel_spmd

    cfg = dict(_CFG)
    if _cfg:
        cfg.update(_cfg)

    orig_shape = np.asarray(input).shape
    in_maps, per = _prepare_in_maps(input, kernel, cfg)
    nc = _get_program(per, cfg)
    res = run_bass_kernel_spmd(
        nc, in_maps, core_ids=list(range(_NCORES)), trace=_trace
    )
    LAST_RESULTS = res
    outs = [res.results[i]["out"] for i in range(_NCORES)]
    return _assemble_output(outs, orig_shape, cfg)
